# revision 18
# baseline (speedup 1.0000x reference)
"""Trainium2 Bass kernel for 5x5 median filter (reflect padding, SAME size).

Input x: [16, 384, 384, 3] f32 (NHWC), k=5. Output: same shape.

Strategy:
- Pure data parallel over 8 NeuronCores: 2 images per core.
- Per core layout: partition p = img*64 + hblock, each hblock = 6 output
  rows. Free dim = (10 input rows) x (68 px * 3 ch) for a 64-px-wide
  chunk (2 px halo each side; channels stay interleaved so horizontal
  pixel shifts are 3-element offsets). 6 chunks cover W=384.
- Median-of-25 via separable sorting network (90 min/max ops/pixel):
  1. vertical sort of 5-row columns (9-CE network, shared across
     horizontal windows)
  2. PM[x] = full Batcher merge of sorted columns (x, x+1) -> sorted 10
  3. per window: L=PM[w-2], R=PM[w+1], M=sorted col w;
     u = 1-idx ranks 8..13 of merge(L,R) via DCE'd Batcher merge(10,10);
     median = 1-idx rank 6 of merge(u, M).
- Reflect padding: row halos via DMAs from reflected rows, column halos
  via on-chip copies at image edges.
"""

import numpy as np

import concourse.bacc as bacc
import concourse.bass as bass
import concourse.mybir as mybir
from concourse.bass_utils import run_bass_kernel_spmd
from concourse.tile import TileContext

f32 = mybir.dt.float32
AMIN = mybir.AluOpType.min
AMAX = mybir.AluOpType.max

H = 384
W = 384
C = 3
ROW = W * C          # 1152 elements per image row
IMG = H * ROW        # elements per image
R = 6                # output rows per partition block
NBLK = H // R        # 64 blocks per image
W_CHUNK = 64         # output px per chunk
N_CHUNK = W // W_CHUNK

WS = (W_CHUNK + 4) * C    # column-sort domain width (els)
WPM = (W_CHUNK + 3) * C   # pair-merge domain width
WSEL = W_CHUNK * C        # selection/output domain width


# ---------------------------------------------------------------------------
# Symbolic min/max DAG with refcounted scratch-tile reuse
# ---------------------------------------------------------------------------

class V:
    __slots__ = ("kind", "op", "a", "b", "w", "tag", "eng", "uses", "ap",
                 "off", "parent")

    def __init__(self, kind, w):
        self.kind = kind      # 'leaf' | 'op' | 'view'
        self.w = w
        self.op = None
        self.a = None
        self.b = None
        self.tag = None
        self.eng = "v"
        self.uses = 0
        self.ap = None
        self.off = 0
        self.parent = None


class Net:
    def __init__(self):
        self.nodes = []

    def leaf(self, ap, w):
        v = V("leaf", w)
        v.ap = ap
        return v

    def _mm(self, op, a, b, tag, eng):
        assert a.w == b.w, (a.w, b.w)
        v = V("op", a.w)
        v.op, v.a, v.b, v.tag = op, a, b, tag
        if eng is not None:
            v.eng = eng
        a.uses += 1
        b.uses += 1
        self.nodes.append(v)
        return v

    def MIN(self, a, b, tag=None, eng=None):
        return self._mm(AMIN, a, b, tag, eng)

    def MAX(self, a, b, tag=None, eng=None):
        return self._mm(AMAX, a, b, tag, eng)

    def CE(self, a, b, tags=(None, None)):
        return self.MIN(a, b, tags[0]), self.MAX(a, b, tags[1])

    def view(self, a, off_el, w):
        v = V("view", w)
        v.parent = a
        v.off = off_el
        a.uses += 1
        return v


class Emitter:
    def __init__(self, nc, pool, n_scratch=12, pool2=None):
        self.engines = {"v": nc.vector, "g": nc.gpsimd, "s": nc.scalar}
        self.pool = pool
        self.pool2 = pool2 or pool   # double-buffered pool for "s*" tags
        self.free = [f"scr{i}" for i in range(n_scratch)]
        self.owner = {}

    def _resolve(self, v):
        if v.kind == "view":
            pap = self._resolve(v.parent)
            return pap[:, :, v.off:v.off + v.w]
        assert v.ap is not None, "operand not yet emitted"
        return v.ap

    def _decref(self, v):
        v.uses -= 1
        assert v.uses >= 0
        if v.uses == 0:
            if v.kind == "view":
                self._decref(v.parent)
            elif v.kind == "op" and v in self.owner:
                self.free.append(self.owner.pop(v))

    def _out_ap(self, v, final_out_ap):
        if final_out_ap is not None:
            return final_out_ap
        if v.tag is not None:
            tag = v.tag
        else:
            assert self.free, "scratch exhausted"
            tag = self.free.pop()
            self.owner[v] = tag
        pool = self.pool2 if tag.startswith("s") and tag[1].isdigit() \
            else self.pool
        t = pool.tile([128, R, v.w], f32, tag=tag, name=tag)
        v.ap = t[:]
        return v.ap

    def _scratch_tile(self, w):
        assert self.free, "scratch exhausted (gp temp)"
        tag = self.free.pop()
        t = self.pool.tile([128, R, w], f32, tag=tag, name=tag)
        return t, tag

    def emit(self, net, final_out_ap=None):
        pairs = find_ce_pairs(net)
        emitted = set()
        last = net.nodes[-1]
        gp = self.engines["g"]
        for v in net.nodes:
            if v in emitted:
                continue
            if v.eng != "g":
                a_ap = self._resolve(v.a)
                b_ap = self._resolve(v.b)
                out_ap = self._out_ap(v, final_out_ap if v is last else None)
                self.engines[v.eng].tensor_tensor(out=out_ap, in0=a_ap,
                                                  in1=b_ap, op=v.op)
                emitted.add(v)
                self._decref(v.a)
                self._decref(v.b)
                continue
            # gpsimd: max(a,b) = a + relu(b-a); min(a,b) = b - relu(b-a)
            partner = pairs.get(v)
            unit = [v]
            if partner is not None and partner.eng == "g" \
                    and partner not in emitted:
                unit.append(partner)
            a_ap = self._resolve(v.a)
            b_ap = self._resolve(v.b)
            d_t, d_tag = self._scratch_tile(v.w)
            gp.tensor_tensor(out=d_t[:], in0=b_ap, in1=a_ap,
                             op=mybir.AluOpType.subtract)
            r_t, r_tag = self._scratch_tile(v.w)
            self.engines["s"].activation(r_t[:], d_t[:],
                                         mybir.ActivationFunctionType.Relu)
            self.free.append(d_tag)
            for u in unit:
                out_ap = self._out_ap(u, final_out_ap if u is last else None)
                if u.op == AMAX:
                    gp.tensor_tensor(out=out_ap, in0=a_ap, in1=r_t[:],
                                     op=mybir.AluOpType.add)
                else:
                    gp.tensor_tensor(out=out_ap, in0=b_ap, in1=r_t[:],
                                     op=mybir.AluOpType.subtract)
                emitted.add(u)
            self.free.append(r_tag)
            for u in unit:
                self._decref(u.a)
                self._decref(u.b)


# ---------------------------------------------------------------------------
# Median network DAG (per chunk)
# ---------------------------------------------------------------------------

def sort5(net, x, tags):
    v = list(x)
    seq = [(0, 1), (3, 4), (2, 4), (2, 3), (1, 4), (0, 3), (0, 2), (1, 3),
           (1, 2)]
    last = {}
    for ni, (i, j) in enumerate(seq):
        last[i] = ni
        last[j] = ni
    for ni, (i, j) in enumerate(seq):
        lo_tag = tags[i] if last[i] == ni else None
        hi_tag = tags[j] if last[j] == ni else None
        v[i], v[j] = net.CE(v[i], v[j], tags=(lo_tag, hi_tag))
    return v


def merge22(net, x0, x1, y0, y1, out_tags=(None, None, None, None)):
    m0 = net.MIN(x0, y0, out_tags[0])
    t = net.MAX(x0, y0)
    s = net.MIN(x1, y1)
    m1 = net.MIN(t, s, out_tags[1])
    m2 = net.MAX(t, s, out_tags[2])
    m3 = net.MAX(x1, y1, out_tags[3])
    return m0, m1, m2, m3


def merge33(net, x0, x1, x2, y0, y1, y2, t0=None, t5=None):
    h0, h1, h2, h3 = merge22(net, x0, x2, y0, y2, (t0, None, None, t5))
    k0 = net.MIN(x1, y1)
    k1 = net.MAX(x1, y1)
    f1 = net.MIN(k0, h1)
    f2 = net.MAX(k0, h1)
    f3 = net.MIN(k1, h2)
    f4 = net.MAX(k1, h2)
    return h0, f1, f2, f3, f4, h3


def merge55(net, a, b, tags):
    f = merge33(net, a[0], a[2], a[4], b[0], b[2], b[4], t0=tags[0],
                t5=tags[9])
    g = merge22(net, a[1], a[3], b[1], b[3])
    out = [f[0]]
    for i in range(4):
        out.append(net.MIN(g[i], f[i + 1], tags[2 * i + 1]))
        out.append(net.MAX(g[i], f[i + 1], tags[2 * i + 2]))
    out.append(f[5])
    return out


def m55_mid_partial(net, A, B, want, tags):
    t1 = net.MAX(A[1], B[1])
    t2 = net.MIN(A[3], B[3])
    g1 = net.MIN(t1, t2)
    g2 = net.MAX(t1, t2)
    k0 = net.MIN(A[2], B[2])
    k1 = net.MAX(A[2], B[2])
    t3 = net.MAX(A[0], B[0])
    t4 = net.MIN(A[4], B[4])
    h1 = net.MIN(t3, t4)
    h2 = net.MAX(t3, t4)
    f2 = net.MAX(k0, h1)
    f3 = net.MIN(k1, h2)
    if want == "o":
        return (net.MIN(g1, f2, tags[0]), net.MAX(g1, f2, tags[1]),
                net.MIN(g2, f3, tags[2]))
    return (net.MAX(g1, f2, tags[0]), net.MIN(g2, f3, tags[1]),
            net.MAX(g2, f3, tags[2]))


# ---------------------------------------------------------------------------
# Kernel builder
# ---------------------------------------------------------------------------

def build_nc():
    nc = bacc.Bacc("TRN2", target_bir_lowering=False)
    x = nc.dram_tensor("x", [2, H, W, C], f32, kind="ExternalInput")
    y = nc.dram_tensor("out", [2, H, W, C], f32, kind="ExternalOutput")

    with TileContext(nc) as tc:
        with tc.tile_pool(name="io", bufs=2) as iop, \
             tc.tile_pool(name="work", bufs=1) as wp:
            for ci in range(N_CHUNK):
                w0 = ci * W_CHUNK
                pxlo = max(0, w0 - 2)
                pxhi = min(W, w0 + W_CHUNK + 2)
                n = (pxhi - pxlo) * C
                elo = (pxlo - (w0 - 2)) * C

                xt = iop.tile([128, 10, WS], f32, tag="xt", name="xt")
                for img in range(2):
                    base = img * IMG + pxlo * C
                    p0 = img * NBLK
                    src = bass.AP(x, base + 4 * ROW,
                                  [[6 * ROW, NBLK - 2], [ROW, 10], [1, n]])
                    nc.sync.dma_start(
                        out=xt[p0 + 1:p0 + NBLK - 1, :, elo:elo + n], in_=src)
                    src = bass.AP(x, base, [[ROW, 1], [ROW, 8], [1, n]])
                    nc.sync.dma_start(out=xt[p0:p0 + 1, 2:10, elo:elo + n],
                                      in_=src)
                    # reflect: j=0 <- row 2, j=1 <- row 1
                    for j, r in ((0, 2), (1, 1)):
                        src = bass.AP(x, base + r * ROW, [[ROW, 1], [1, n]])
                        nc.sync.dma_start(
                            out=xt[p0:p0 + 1, j:j + 1, elo:elo + n], in_=src)
                    p63 = p0 + NBLK - 1
                    src = bass.AP(x, base + 376 * ROW,
                                  [[ROW, 1], [ROW, 8], [1, n]])
                    nc.sync.dma_start(out=xt[p63:p63 + 1, 0:8, elo:elo + n],
                                      in_=src)
                    # reflect: j=8 <- row 382, j=9 <- row 381
                    for j, r in ((8, 382), (9, 381)):
                        src = bass.AP(x, base + r * ROW, [[ROW, 1], [1, n]])
                        nc.sync.dma_start(
                            out=xt[p63:p63 + 1, j:j + 1, elo:elo + n],
                            in_=src)

                if ci == 0:
                    # col -2 <- col 2 (els 12:15 -> 0:3); col -1 <- col 1
                    nc.scalar.copy(out=xt[:, :, 0:C],
                                   in_=xt[:, :, 4 * C:5 * C])
                    nc.scalar.copy(out=xt[:, :, C:2 * C],
                                   in_=xt[:, :, 3 * C:4 * C])
                if ci == N_CHUNK - 1:
                    # col 384 <- col 382 (t66 <- t64); col 385 <- col 381
                    nc.scalar.copy(out=xt[:, :, 66 * C:67 * C],
                                   in_=xt[:, :, 64 * C:65 * C])
                    nc.scalar.copy(out=xt[:, :, 67 * C:68 * C],
                                   in_=xt[:, :, 63 * C:64 * C])

                outt = iop.tile([128, R, WSEL], f32, tag="outt", name="outt")

                net = build_chunk_net_real(xt)
                assign_engines(net, enable_gp=False)
                em = Emitter(nc, wp, pool2=iop)
                em.emit(net, final_out_ap=outt[:])

                for img in range(2):
                    p0 = img * NBLK
                    dst = bass.AP(y, img * IMG + w0 * C,
                                  [[R * ROW, NBLK], [ROW, R], [1, WSEL]])
                    nc.sync.dma_start(out=dst, in_=outt[p0:p0 + NBLK, :, :])

    nc.finalize()
    return nc


def build_chunk_net_real(xt):
    net = Net()
    leaves = [net.leaf(xt[:, d:d + R, :], WS) for d in range(5)]
    # inline build (same as build_chunk_net but with shared net)
    s = sort5(net, leaves, {i: f"s{i}" for i in range(5)})
    a = [net.view(s[i], 0, WPM) for i in range(5)]
    b = [net.view(s[i], C, WPM) for i in range(5)]
    pm = merge55(net, a, b, [f"pm{i}" for i in range(10)])
    Lv = [net.view(p, 0, WSEL) for p in pm]
    Rv = [net.view(p, 3 * C, WSEL) for p in pm]
    M = [net.view(s[i], 2 * C, WSEL) for i in range(5)]
    o3, o4, o5 = m55_mid_partial(net, Lv[1::2], Rv[1::2], "o",
                                 ["oe0", "oe1", "oe2"])
    e4, e5, e6 = m55_mid_partial(net, Lv[0::2], Rv[0::2], "e",
                                 ["oe3", "oe4", "oe5"])
    u0 = net.MIN(o3, e4, "pm0")
    u1 = net.MAX(o3, e4, "pm1")
    u2 = net.MIN(o4, e5, "pm2")
    u3 = net.MAX(o4, e5, "pm3")
    u4 = net.MIN(o5, e6, "pm4")
    u5 = net.MAX(o5, e6, "pm5")
    q0 = net.MIN(u3, M[3])
    p1 = net.MIN(net.MAX(u1, M[1]), u5)
    o2p = net.MAX(q0, p1)
    k1p = net.MAX(u2, M[2])
    h2p = net.MAX(net.MAX(u0, M[0]), net.MIN(u4, M[4]))
    e3p = net.MIN(k1p, h2p)
    net.MIN(o2p, e3p)
    return net


def find_ce_pairs(net):
    """Detect (min, max) node pairs on identical operands (CE pairs).
    Returns dict node -> partner (both directions)."""
    pairs = {}
    by_key = {}
    for v in net.nodes:
        key = (id(v.a), id(v.b))
        if key in by_key:
            u = by_key[key]
            if u.op != v.op and u not in pairs:
                pairs[u] = v
                pairs[v] = u
                continue
        by_key[key] = v
    return pairs


def assign_engines(net, pair_gp=3.61, single_gp=5.13, enable_gp=True):
    """Greedy two-engine list scheduling over schedulable units (CE pairs
    merged). Costs in DVE-op units. Mutates node.eng."""
    pairs = find_ce_pairs(net)
    avail = {"v": 0.0, "g": 0.0}
    done = {}

    def ready(v):
        if v.kind == "leaf":
            return 0.0
        if v.kind == "view":
            return ready(v.parent)
        return done[v]

    seen = set()
    for v in net.nodes:
        if v in seen:
            continue
        partner = pairs.get(v)
        if partner is not None:
            unit = (v, partner)
            cost_v, cost_g = 2.0, pair_gp
        else:
            unit = (v,)
            cost_v, cost_g = 1.0, single_gp
        dep = 0.0
        for u in unit:
            dep = max(dep, ready(u.a), ready(u.b))
        fin_v = max(avail["v"], dep) + cost_v
        fin_g = max(avail["g"], dep) + cost_g
        if enable_gp and fin_g < fin_v:
            eng, fin = "g", fin_g
        else:
            eng, fin = "v", fin_v
        for u in unit:
            u.eng = eng
            done[u] = fin
            seen.add(u)
        avail[eng] = fin
    return avail


_NC = None


def _get_nc():
    global _NC
    if _NC is None:
        _NC = build_nc()
    return _NC


def kernel(x, k):
    assert int(k) == 5
    x = np.ascontiguousarray(np.asarray(x, dtype=np.float32))
    assert x.shape == (16, H, W, C)
    nc = _get_nc()
    in_maps = [{"x": x[2 * i:2 * i + 2]} for i in range(8)]
    res = run_bass_kernel_spmd(nc, in_maps, core_ids=list(range(8)))
    return np.concatenate([r["out"] for r in res.results], axis=0)


# revision 19
# speedup vs baseline: 1.0097x; 1.0097x over previous
"""Trainium2 Bass kernel for 5x5 median filter (reflect padding, SAME size).

Input x: [16, 384, 384, 3] f32 (NHWC), k=5. Output: same shape.

Strategy:
- Pure data parallel over 8 NeuronCores: 2 images per core.
- Per core layout: partition p = img*64 + hblock, each hblock = 6 output
  rows. Free dim = (10 input rows) x (68 px * 3 ch) for a 64-px-wide
  chunk (2 px halo each side; channels stay interleaved so horizontal
  pixel shifts are 3-element offsets). 6 chunks cover W=384.
- Median-of-25 via separable sorting network (90 min/max ops/pixel):
  1. vertical sort of 5-row columns (9-CE network, shared across
     horizontal windows)
  2. PM[x] = full Batcher merge of sorted columns (x, x+1) -> sorted 10
  3. per window: L=PM[w-2], R=PM[w+1], M=sorted col w;
     u = 1-idx ranks 8..13 of merge(L,R) via DCE'd Batcher merge(10,10);
     median = 1-idx rank 6 of merge(u, M).
- Reflect padding: row halos via DMAs from reflected rows, column halos
  via on-chip copies at image edges.
"""

import numpy as np

import concourse.bacc as bacc
import concourse.bass as bass
import concourse.mybir as mybir
from concourse.bass_utils import run_bass_kernel_spmd
from concourse.tile import TileContext

f32 = mybir.dt.float32
AMIN = mybir.AluOpType.min
AMAX = mybir.AluOpType.max

H = 384
W = 384
C = 3
ROW = W * C          # 1152 elements per image row
IMG = H * ROW        # elements per image
R = 6                # output rows per partition block
NBLK = H // R        # 64 blocks per image
W_CHUNK = 96         # output px per chunk
N_CHUNK = W // W_CHUNK

WS = (W_CHUNK + 4) * C    # column-sort domain width (els)
WPM = (W_CHUNK + 3) * C   # pair-merge domain width
WSEL = W_CHUNK * C        # selection/output domain width


# ---------------------------------------------------------------------------
# Symbolic min/max DAG with refcounted scratch-tile reuse
# ---------------------------------------------------------------------------

class V:
    __slots__ = ("kind", "op", "a", "b", "w", "tag", "eng", "uses", "ap",
                 "off", "parent")

    def __init__(self, kind, w):
        self.kind = kind      # 'leaf' | 'op' | 'view'
        self.w = w
        self.op = None
        self.a = None
        self.b = None
        self.tag = None
        self.eng = "v"
        self.uses = 0
        self.ap = None
        self.off = 0
        self.parent = None


class Net:
    def __init__(self):
        self.nodes = []

    def leaf(self, ap, w):
        v = V("leaf", w)
        v.ap = ap
        return v

    def _mm(self, op, a, b, tag, eng):
        assert a.w == b.w, (a.w, b.w)
        v = V("op", a.w)
        v.op, v.a, v.b, v.tag = op, a, b, tag
        if eng is not None:
            v.eng = eng
        a.uses += 1
        b.uses += 1
        self.nodes.append(v)
        return v

    def MIN(self, a, b, tag=None, eng=None):
        return self._mm(AMIN, a, b, tag, eng)

    def MAX(self, a, b, tag=None, eng=None):
        return self._mm(AMAX, a, b, tag, eng)

    def CE(self, a, b, tags=(None, None)):
        return self.MIN(a, b, tags[0]), self.MAX(a, b, tags[1])

    def view(self, a, off_el, w):
        v = V("view", w)
        v.parent = a
        v.off = off_el
        a.uses += 1
        return v


class Emitter:
    def __init__(self, nc, pool, n_scratch=12, pool2=None):
        self.engines = {"v": nc.vector, "g": nc.gpsimd, "s": nc.scalar}
        self.pool = pool
        self.pool2 = pool2 or pool   # double-buffered pool for "s*" tags
        self.free = [f"scr{i}" for i in range(n_scratch)]
        self.owner = {}

    def _resolve(self, v):
        if v.kind == "view":
            pap = self._resolve(v.parent)
            return pap[:, :, v.off:v.off + v.w]
        assert v.ap is not None, "operand not yet emitted"
        return v.ap

    def _decref(self, v):
        v.uses -= 1
        assert v.uses >= 0
        if v.uses == 0:
            if v.kind == "view":
                self._decref(v.parent)
            elif v.kind == "op" and v in self.owner:
                self.free.append(self.owner.pop(v))

    def _out_ap(self, v, final_out_ap):
        if final_out_ap is not None:
            return final_out_ap
        if v.tag is not None:
            tag = v.tag
        else:
            assert self.free, "scratch exhausted"
            tag = self.free.pop()
            self.owner[v] = tag
        pool = self.pool2 if tag.startswith("s") and tag[1].isdigit() \
            else self.pool
        t = pool.tile([128, R, v.w], f32, tag=tag, name=tag)
        v.ap = t[:]
        return v.ap

    def _scratch_tile(self, w):
        assert self.free, "scratch exhausted (gp temp)"
        tag = self.free.pop()
        t = self.pool.tile([128, R, w], f32, tag=tag, name=tag)
        return t, tag

    def emit(self, net, final_out_ap=None):
        pairs = find_ce_pairs(net)
        emitted = set()
        last = net.nodes[-1]
        gp = self.engines["g"]
        for v in net.nodes:
            if v in emitted:
                continue
            if v.eng != "g":
                a_ap = self._resolve(v.a)
                b_ap = self._resolve(v.b)
                out_ap = self._out_ap(v, final_out_ap if v is last else None)
                self.engines[v.eng].tensor_tensor(out=out_ap, in0=a_ap,
                                                  in1=b_ap, op=v.op)
                emitted.add(v)
                self._decref(v.a)
                self._decref(v.b)
                continue
            # gpsimd: max(a,b) = a + relu(b-a); min(a,b) = b - relu(b-a)
            partner = pairs.get(v)
            unit = [v]
            if partner is not None and partner.eng == "g" \
                    and partner not in emitted:
                unit.append(partner)
            a_ap = self._resolve(v.a)
            b_ap = self._resolve(v.b)
            d_t, d_tag = self._scratch_tile(v.w)
            gp.tensor_tensor(out=d_t[:], in0=b_ap, in1=a_ap,
                             op=mybir.AluOpType.subtract)
            r_t, r_tag = self._scratch_tile(v.w)
            self.engines["s"].activation(r_t[:], d_t[:],
                                         mybir.ActivationFunctionType.Relu)
            self.free.append(d_tag)
            for u in unit:
                out_ap = self._out_ap(u, final_out_ap if u is last else None)
                if u.op == AMAX:
                    gp.tensor_tensor(out=out_ap, in0=a_ap, in1=r_t[:],
                                     op=mybir.AluOpType.add)
                else:
                    gp.tensor_tensor(out=out_ap, in0=b_ap, in1=r_t[:],
                                     op=mybir.AluOpType.subtract)
                emitted.add(u)
            self.free.append(r_tag)
            for u in unit:
                self._decref(u.a)
                self._decref(u.b)


# ---------------------------------------------------------------------------
# Median network DAG (per chunk)
# ---------------------------------------------------------------------------

def sort5(net, x, tags):
    v = list(x)
    seq = [(0, 1), (3, 4), (2, 4), (2, 3), (1, 4), (0, 3), (0, 2), (1, 3),
           (1, 2)]
    last = {}
    for ni, (i, j) in enumerate(seq):
        last[i] = ni
        last[j] = ni
    for ni, (i, j) in enumerate(seq):
        lo_tag = tags[i] if last[i] == ni else None
        hi_tag = tags[j] if last[j] == ni else None
        v[i], v[j] = net.CE(v[i], v[j], tags=(lo_tag, hi_tag))
    return v


def merge22(net, x0, x1, y0, y1, out_tags=(None, None, None, None)):
    m0 = net.MIN(x0, y0, out_tags[0])
    t = net.MAX(x0, y0)
    s = net.MIN(x1, y1)
    m1 = net.MIN(t, s, out_tags[1])
    m2 = net.MAX(t, s, out_tags[2])
    m3 = net.MAX(x1, y1, out_tags[3])
    return m0, m1, m2, m3


def merge33(net, x0, x1, x2, y0, y1, y2, t0=None, t5=None):
    h0, h1, h2, h3 = merge22(net, x0, x2, y0, y2, (t0, None, None, t5))
    k0 = net.MIN(x1, y1)
    k1 = net.MAX(x1, y1)
    f1 = net.MIN(k0, h1)
    f2 = net.MAX(k0, h1)
    f3 = net.MIN(k1, h2)
    f4 = net.MAX(k1, h2)
    return h0, f1, f2, f3, f4, h3


def merge55(net, a, b, tags):
    f = merge33(net, a[0], a[2], a[4], b[0], b[2], b[4], t0=tags[0],
                t5=tags[9])
    g = merge22(net, a[1], a[3], b[1], b[3])
    out = [f[0]]
    for i in range(4):
        out.append(net.MIN(g[i], f[i + 1], tags[2 * i + 1]))
        out.append(net.MAX(g[i], f[i + 1], tags[2 * i + 2]))
    out.append(f[5])
    return out


def m55_mid_partial(net, A, B, want, tags):
    t1 = net.MAX(A[1], B[1])
    t2 = net.MIN(A[3], B[3])
    g1 = net.MIN(t1, t2)
    g2 = net.MAX(t1, t2)
    k0 = net.MIN(A[2], B[2])
    k1 = net.MAX(A[2], B[2])
    t3 = net.MAX(A[0], B[0])
    t4 = net.MIN(A[4], B[4])
    h1 = net.MIN(t3, t4)
    h2 = net.MAX(t3, t4)
    f2 = net.MAX(k0, h1)
    f3 = net.MIN(k1, h2)
    if want == "o":
        return (net.MIN(g1, f2, tags[0]), net.MAX(g1, f2, tags[1]),
                net.MIN(g2, f3, tags[2]))
    return (net.MAX(g1, f2, tags[0]), net.MIN(g2, f3, tags[1]),
            net.MAX(g2, f3, tags[2]))


# ---------------------------------------------------------------------------
# Kernel builder
# ---------------------------------------------------------------------------

def build_nc():
    nc = bacc.Bacc("TRN2", target_bir_lowering=False)
    x = nc.dram_tensor("x", [2, H, W, C], f32, kind="ExternalInput")
    y = nc.dram_tensor("out", [2, H, W, C], f32, kind="ExternalOutput")

    with TileContext(nc) as tc:
        with tc.tile_pool(name="io", bufs=2) as iop, \
             tc.tile_pool(name="work", bufs=1) as wp:
            for ci in range(N_CHUNK):
                w0 = ci * W_CHUNK
                pxlo = max(0, w0 - 2)
                pxhi = min(W, w0 + W_CHUNK + 2)
                n = (pxhi - pxlo) * C
                elo = (pxlo - (w0 - 2)) * C

                xt = iop.tile([128, 10, WS], f32, tag="xt", name="xt")
                for img in range(2):
                    base = img * IMG + pxlo * C
                    p0 = img * NBLK
                    src = bass.AP(x, base + 4 * ROW,
                                  [[6 * ROW, NBLK - 2], [ROW, 10], [1, n]])
                    nc.sync.dma_start(
                        out=xt[p0 + 1:p0 + NBLK - 1, :, elo:elo + n], in_=src)
                    src = bass.AP(x, base, [[ROW, 1], [ROW, 8], [1, n]])
                    nc.sync.dma_start(out=xt[p0:p0 + 1, 2:10, elo:elo + n],
                                      in_=src)
                    # reflect: j=0 <- row 2, j=1 <- row 1
                    for j, r in ((0, 2), (1, 1)):
                        src = bass.AP(x, base + r * ROW, [[ROW, 1], [1, n]])
                        nc.sync.dma_start(
                            out=xt[p0:p0 + 1, j:j + 1, elo:elo + n], in_=src)
                    p63 = p0 + NBLK - 1
                    src = bass.AP(x, base + 376 * ROW,
                                  [[ROW, 1], [ROW, 8], [1, n]])
                    nc.sync.dma_start(out=xt[p63:p63 + 1, 0:8, elo:elo + n],
                                      in_=src)
                    # reflect: j=8 <- row 382, j=9 <- row 381
                    for j, r in ((8, 382), (9, 381)):
                        src = bass.AP(x, base + r * ROW, [[ROW, 1], [1, n]])
                        nc.sync.dma_start(
                            out=xt[p63:p63 + 1, j:j + 1, elo:elo + n],
                            in_=src)

                if ci == 0:
                    # col -2 <- col 2 (els 12:15 -> 0:3); col -1 <- col 1
                    nc.scalar.copy(out=xt[:, :, 0:C],
                                   in_=xt[:, :, 4 * C:5 * C])
                    nc.scalar.copy(out=xt[:, :, C:2 * C],
                                   in_=xt[:, :, 3 * C:4 * C])
                if ci == N_CHUNK - 1:
                    # col W <- col W-2 ; col W+1 <- col W-3
                    wc = W_CHUNK
                    nc.scalar.copy(out=xt[:, :, (wc + 2) * C:(wc + 3) * C],
                                   in_=xt[:, :, wc * C:(wc + 1) * C])
                    nc.scalar.copy(out=xt[:, :, (wc + 3) * C:(wc + 4) * C],
                                   in_=xt[:, :, (wc - 1) * C:wc * C])

                outt = iop.tile([128, R, WSEL], f32, tag="outt", name="outt")

                net = build_chunk_net_real(xt)
                assign_engines(net, enable_gp=False)
                em = Emitter(nc, wp, n_scratch=9)
                em.emit(net, final_out_ap=outt[:])

                for img in range(2):
                    p0 = img * NBLK
                    dst = bass.AP(y, img * IMG + w0 * C,
                                  [[R * ROW, NBLK], [ROW, R], [1, WSEL]])
                    nc.sync.dma_start(out=dst, in_=outt[p0:p0 + NBLK, :, :])

    nc.finalize()
    return nc


def build_chunk_net_real(xt):
    net = Net()
    leaves = [net.leaf(xt[:, d:d + R, :], WS) for d in range(5)]
    # inline build (same as build_chunk_net but with shared net)
    s = sort5(net, leaves, {i: f"s{i}" for i in range(5)})
    a = [net.view(s[i], 0, WPM) for i in range(5)]
    b = [net.view(s[i], C, WPM) for i in range(5)]
    pm = merge55(net, a, b, [f"pm{i}" for i in range(10)])
    Lv = [net.view(p, 0, WSEL) for p in pm]
    Rv = [net.view(p, 3 * C, WSEL) for p in pm]
    M = [net.view(s[i], 2 * C, WSEL) for i in range(5)]
    # o/e outputs and u reuse dead PM slots (odd slots die after the
    # o-partial internals, even slots after the e-partial internals)
    o3, o4, o5 = m55_mid_partial(net, Lv[1::2], Rv[1::2], "o",
                                 ["pm1", "pm3", "pm5"])
    e4, e5, e6 = m55_mid_partial(net, Lv[0::2], Rv[0::2], "e",
                                 ["pm7", "pm9", "pm0"])
    u0 = net.MIN(o3, e4, "pm2")
    u1 = net.MAX(o3, e4, "pm4")
    u2 = net.MIN(o4, e5, "pm6")
    u3 = net.MAX(o4, e5, "pm8")
    u4 = net.MIN(o5, e6)
    u5 = net.MAX(o5, e6)
    q0 = net.MIN(u3, M[3])
    p1 = net.MIN(net.MAX(u1, M[1]), u5)
    o2p = net.MAX(q0, p1)
    k1p = net.MAX(u2, M[2])
    h2p = net.MAX(net.MAX(u0, M[0]), net.MIN(u4, M[4]))
    e3p = net.MIN(k1p, h2p)
    net.MIN(o2p, e3p)
    return net


def find_ce_pairs(net):
    """Detect (min, max) node pairs on identical operands (CE pairs).
    Returns dict node -> partner (both directions)."""
    pairs = {}
    by_key = {}
    for v in net.nodes:
        key = (id(v.a), id(v.b))
        if key in by_key:
            u = by_key[key]
            if u.op != v.op and u not in pairs:
                pairs[u] = v
                pairs[v] = u
                continue
        by_key[key] = v
    return pairs


def assign_engines(net, pair_gp=3.61, single_gp=5.13, enable_gp=True):
    """Greedy two-engine list scheduling over schedulable units (CE pairs
    merged). Costs in DVE-op units. Mutates node.eng."""
    pairs = find_ce_pairs(net)
    avail = {"v": 0.0, "g": 0.0}
    done = {}

    def ready(v):
        if v.kind == "leaf":
            return 0.0
        if v.kind == "view":
            return ready(v.parent)
        return done[v]

    seen = set()
    for v in net.nodes:
        if v in seen:
            continue
        partner = pairs.get(v)
        if partner is not None:
            unit = (v, partner)
            cost_v, cost_g = 2.0, pair_gp
        else:
            unit = (v,)
            cost_v, cost_g = 1.0, single_gp
        dep = 0.0
        for u in unit:
            dep = max(dep, ready(u.a), ready(u.b))
        fin_v = max(avail["v"], dep) + cost_v
        fin_g = max(avail["g"], dep) + cost_g
        if enable_gp and fin_g < fin_v:
            eng, fin = "g", fin_g
        else:
            eng, fin = "v", fin_v
        for u in unit:
            u.eng = eng
            done[u] = fin
            seen.add(u)
        avail[eng] = fin
    return avail


_NC = None


def _get_nc():
    global _NC
    if _NC is None:
        _NC = build_nc()
    return _NC


def kernel(x, k):
    assert int(k) == 5
    x = np.ascontiguousarray(np.asarray(x, dtype=np.float32))
    assert x.shape == (16, H, W, C)
    nc = _get_nc()
    in_maps = [{"x": x[2 * i:2 * i + 2]} for i in range(8)]
    res = run_bass_kernel_spmd(nc, in_maps, core_ids=list(range(8)))
    return np.concatenate([r["out"] for r in res.results], axis=0)


# revision 20
# speedup vs baseline: 1.0526x; 1.0424x over previous
"""Trainium2 Bass kernel for 5x5 median filter (reflect padding, SAME size).

Input x: [16, 384, 384, 3] f32 (NHWC), k=5. Output: same shape.

Strategy:
- Pure data parallel over 8 NeuronCores: 2 images per core.
- Per core layout: partition p = img*64 + hblock, each hblock = 6 output
  rows. Free dim = (10 input rows) x (68 px * 3 ch) for a 64-px-wide
  chunk (2 px halo each side; channels stay interleaved so horizontal
  pixel shifts are 3-element offsets). 6 chunks cover W=384.
- Median-of-25 via separable sorting network (90 min/max ops/pixel):
  1. vertical sort of 5-row columns (9-CE network, shared across
     horizontal windows)
  2. PM[x] = full Batcher merge of sorted columns (x, x+1) -> sorted 10
  3. per window: L=PM[w-2], R=PM[w+1], M=sorted col w;
     u = 1-idx ranks 8..13 of merge(L,R) via DCE'd Batcher merge(10,10);
     median = 1-idx rank 6 of merge(u, M).
- Reflect padding: row halos via DMAs from reflected rows, column halos
  via on-chip copies at image edges.
"""

import numpy as np

import concourse.bacc as bacc
import concourse.bass as bass
import concourse.mybir as mybir
from concourse.bass_utils import run_bass_kernel_spmd
from concourse.tile import TileContext

f32 = mybir.dt.float32
AMIN = mybir.AluOpType.min
AMAX = mybir.AluOpType.max

H = 384
W = 384
C = 3
ROW = W * C          # 1152 elements per image row
IMG = H * ROW        # elements per image
R = 6                # output rows per partition block
NBLK = H // R        # 64 blocks per image
W_CHUNK = 96         # output px per chunk
N_CHUNK = W // W_CHUNK

WS = (W_CHUNK + 4) * C    # column-sort domain width (els)
WPM = (W_CHUNK + 3) * C   # pair-merge domain width
WSEL = W_CHUNK * C        # selection/output domain width


# ---------------------------------------------------------------------------
# Symbolic min/max DAG with refcounted scratch-tile reuse
# ---------------------------------------------------------------------------

class V:
    __slots__ = ("kind", "op", "a", "b", "w", "tag", "eng", "uses", "ap",
                 "off", "parent")

    def __init__(self, kind, w):
        self.kind = kind      # 'leaf' | 'op' | 'view'
        self.w = w
        self.op = None
        self.a = None
        self.b = None
        self.tag = None
        self.eng = "v"
        self.uses = 0
        self.ap = None
        self.off = 0
        self.parent = None


class Net:
    def __init__(self):
        self.nodes = []

    def leaf(self, ap, w):
        v = V("leaf", w)
        v.ap = ap
        return v

    def _mm(self, op, a, b, tag, eng):
        assert a.w == b.w, (a.w, b.w)
        v = V("op", a.w)
        v.op, v.a, v.b, v.tag = op, a, b, tag
        if eng is not None:
            v.eng = eng
        a.uses += 1
        b.uses += 1
        self.nodes.append(v)
        return v

    def MIN(self, a, b, tag=None, eng=None):
        return self._mm(AMIN, a, b, tag, eng)

    def MAX(self, a, b, tag=None, eng=None):
        return self._mm(AMAX, a, b, tag, eng)

    def CE(self, a, b, tags=(None, None)):
        return self.MIN(a, b, tags[0]), self.MAX(a, b, tags[1])

    def view(self, a, off_el, w):
        v = V("view", w)
        v.parent = a
        v.off = off_el
        a.uses += 1
        return v


class Emitter:
    def __init__(self, nc, pool, n_scratch=12, pool2=None):
        self.engines = {"v": nc.vector, "g": nc.gpsimd, "s": nc.scalar}
        self.pool = pool
        self.pool2 = pool2 or pool   # double-buffered pool for "s*" tags
        self.free = [f"scr{i}" for i in range(n_scratch)]
        self.owner = {}

    def _resolve(self, v):
        if v.kind == "view":
            pap = self._resolve(v.parent)
            return pap[:, :, v.off:v.off + v.w]
        assert v.ap is not None, "operand not yet emitted"
        return v.ap

    def _decref(self, v):
        v.uses -= 1
        assert v.uses >= 0
        if v.uses == 0:
            if v.kind == "view":
                self._decref(v.parent)
            elif v.kind == "op" and v in self.owner:
                self.free.append(self.owner.pop(v))

    def _out_ap(self, v, final_out_ap):
        if final_out_ap is not None:
            return final_out_ap
        if v.tag is not None:
            tag = v.tag
        else:
            assert self.free, "scratch exhausted"
            tag = self.free.pop()
            self.owner[v] = tag
        pool = self.pool2 if tag.startswith("s") and tag[1].isdigit() \
            else self.pool
        t = pool.tile([128, R, v.w], f32, tag=tag, name=tag)
        v.ap = t[:]
        return v.ap

    def _scratch_tile(self, w):
        assert self.free, "scratch exhausted (gp temp)"
        tag = self.free.pop()
        t = self.pool.tile([128, R, w], f32, tag=tag, name=tag)
        return t, tag

    def emit(self, net, final_out_ap=None):
        pairs = find_ce_pairs(net)
        emitted = set()
        last = net.nodes[-1]
        gp = self.engines["g"]
        for v in net.nodes:
            if v in emitted:
                continue
            if v.eng != "g":
                a_ap = self._resolve(v.a)
                b_ap = self._resolve(v.b)
                out_ap = self._out_ap(v, final_out_ap if v is last else None)
                self.engines[v.eng].tensor_tensor(out=out_ap, in0=a_ap,
                                                  in1=b_ap, op=v.op)
                emitted.add(v)
                self._decref(v.a)
                self._decref(v.b)
                continue
            # gpsimd: max(a,b) = a + relu(b-a); min(a,b) = b - relu(b-a)
            partner = pairs.get(v)
            unit = [v]
            if partner is not None and partner.eng == "g" \
                    and partner not in emitted:
                unit.append(partner)
            a_ap = self._resolve(v.a)
            b_ap = self._resolve(v.b)
            d_t, d_tag = self._scratch_tile(v.w)
            gp.tensor_tensor(out=d_t[:], in0=b_ap, in1=a_ap,
                             op=mybir.AluOpType.subtract)
            r_t, r_tag = self._scratch_tile(v.w)
            self.engines["s"].activation(r_t[:], d_t[:],
                                         mybir.ActivationFunctionType.Relu)
            self.free.append(d_tag)
            for u in unit:
                out_ap = self._out_ap(u, final_out_ap if u is last else None)
                if u.op == AMAX:
                    gp.tensor_tensor(out=out_ap, in0=a_ap, in1=r_t[:],
                                     op=mybir.AluOpType.add)
                else:
                    gp.tensor_tensor(out=out_ap, in0=b_ap, in1=r_t[:],
                                     op=mybir.AluOpType.subtract)
                emitted.add(u)
            self.free.append(r_tag)
            for u in unit:
                self._decref(u.a)
                self._decref(u.b)


# ---------------------------------------------------------------------------
# Median network DAG (per chunk)
# ---------------------------------------------------------------------------

def sort5(net, x, tags):
    v = list(x)
    seq = [(0, 1), (3, 4), (2, 4), (2, 3), (1, 4), (0, 3), (0, 2), (1, 3),
           (1, 2)]
    last = {}
    for ni, (i, j) in enumerate(seq):
        last[i] = ni
        last[j] = ni
    for ni, (i, j) in enumerate(seq):
        lo_tag = tags[i] if last[i] == ni else None
        hi_tag = tags[j] if last[j] == ni else None
        v[i], v[j] = net.CE(v[i], v[j], tags=(lo_tag, hi_tag))
    return v


def merge22(net, x0, x1, y0, y1, out_tags=(None, None, None, None)):
    m0 = net.MIN(x0, y0, out_tags[0])
    t = net.MAX(x0, y0)
    s = net.MIN(x1, y1)
    m1 = net.MIN(t, s, out_tags[1])
    m2 = net.MAX(t, s, out_tags[2])
    m3 = net.MAX(x1, y1, out_tags[3])
    return m0, m1, m2, m3


def merge33(net, x0, x1, x2, y0, y1, y2, t0=None, t5=None):
    h0, h1, h2, h3 = merge22(net, x0, x2, y0, y2, (t0, None, None, t5))
    k0 = net.MIN(x1, y1)
    k1 = net.MAX(x1, y1)
    f1 = net.MIN(k0, h1)
    f2 = net.MAX(k0, h1)
    f3 = net.MIN(k1, h2)
    f4 = net.MAX(k1, h2)
    return h0, f1, f2, f3, f4, h3


def merge55(net, a, b, tags):
    f = merge33(net, a[0], a[2], a[4], b[0], b[2], b[4], t0=tags[0],
                t5=tags[9])
    g = merge22(net, a[1], a[3], b[1], b[3])
    out = [f[0]]
    for i in range(4):
        out.append(net.MIN(g[i], f[i + 1], tags[2 * i + 1]))
        out.append(net.MAX(g[i], f[i + 1], tags[2 * i + 2]))
    out.append(f[5])
    return out


def m55_mid_partial(net, A, B, want, tags):
    t1 = net.MAX(A[1], B[1])
    t2 = net.MIN(A[3], B[3])
    g1 = net.MIN(t1, t2)
    g2 = net.MAX(t1, t2)
    k0 = net.MIN(A[2], B[2])
    k1 = net.MAX(A[2], B[2])
    t3 = net.MAX(A[0], B[0])
    t4 = net.MIN(A[4], B[4])
    h1 = net.MIN(t3, t4)
    h2 = net.MAX(t3, t4)
    f2 = net.MAX(k0, h1)
    f3 = net.MIN(k1, h2)
    if want == "o":
        return (net.MIN(g1, f2, tags[0]), net.MAX(g1, f2, tags[1]),
                net.MIN(g2, f3, tags[2]))
    return (net.MAX(g1, f2, tags[0]), net.MIN(g2, f3, tags[1]),
            net.MAX(g2, f3, tags[2]))


# ---------------------------------------------------------------------------
# Kernel builder
# ---------------------------------------------------------------------------

def build_nc():
    nc = bacc.Bacc("TRN2", target_bir_lowering=False)
    x = nc.dram_tensor("x", [2, H, W, C], f32, kind="ExternalInput")
    y = nc.dram_tensor("out", [2, H, W, C], f32, kind="ExternalOutput")

    with TileContext(nc) as tc:
        with tc.tile_pool(name="io", bufs=2) as iop, \
             tc.tile_pool(name="work", bufs=1) as wp:
            for ci in range(N_CHUNK):
                w0 = ci * W_CHUNK
                pxlo = max(0, w0 - 2)
                pxhi = min(W, w0 + W_CHUNK + 2)
                n = (pxhi - pxlo) * C
                elo = (pxlo - (w0 - 2)) * C

                xt = iop.tile([128, 10, WS], f32, tag="xt", name="xt")
                for img in range(2):
                    base = img * IMG + pxlo * C
                    p0 = img * NBLK
                    # interior blocks hb=1..62 split into 4 DMAs so they
                    # spread across DMA queues (cuts first-chunk latency)
                    splits = [1, 17, 33, 48, 63]
                    for si in range(4):
                        h0, h1 = splits[si], splits[si + 1]
                        src = bass.AP(x, base + (6 * h0 - 2) * ROW,
                                      [[6 * ROW, h1 - h0], [ROW, 10], [1, n]])
                        nc.sync.dma_start(
                            out=xt[p0 + h0:p0 + h1, :, elo:elo + n], in_=src)
                    src = bass.AP(x, base, [[ROW, 1], [ROW, 8], [1, n]])
                    nc.sync.dma_start(out=xt[p0:p0 + 1, 2:10, elo:elo + n],
                                      in_=src)
                    # reflect: j=0 <- row 2, j=1 <- row 1
                    for j, r in ((0, 2), (1, 1)):
                        src = bass.AP(x, base + r * ROW, [[ROW, 1], [1, n]])
                        nc.sync.dma_start(
                            out=xt[p0:p0 + 1, j:j + 1, elo:elo + n], in_=src)
                    p63 = p0 + NBLK - 1
                    src = bass.AP(x, base + 376 * ROW,
                                  [[ROW, 1], [ROW, 8], [1, n]])
                    nc.sync.dma_start(out=xt[p63:p63 + 1, 0:8, elo:elo + n],
                                      in_=src)
                    # reflect: j=8 <- row 382, j=9 <- row 381
                    for j, r in ((8, 382), (9, 381)):
                        src = bass.AP(x, base + r * ROW, [[ROW, 1], [1, n]])
                        nc.sync.dma_start(
                            out=xt[p63:p63 + 1, j:j + 1, elo:elo + n],
                            in_=src)

                if ci == 0:
                    # col -2 <- col 2 (els 12:15 -> 0:3); col -1 <- col 1
                    nc.scalar.copy(out=xt[:, :, 0:C],
                                   in_=xt[:, :, 4 * C:5 * C])
                    nc.scalar.copy(out=xt[:, :, C:2 * C],
                                   in_=xt[:, :, 3 * C:4 * C])
                if ci == N_CHUNK - 1:
                    # col W <- col W-2 ; col W+1 <- col W-3
                    wc = W_CHUNK
                    nc.scalar.copy(out=xt[:, :, (wc + 2) * C:(wc + 3) * C],
                                   in_=xt[:, :, wc * C:(wc + 1) * C])
                    nc.scalar.copy(out=xt[:, :, (wc + 3) * C:(wc + 4) * C],
                                   in_=xt[:, :, (wc - 1) * C:wc * C])

                outt = iop.tile([128, R, WSEL], f32, tag="outt", name="outt")

                net = build_chunk_net_real(xt)
                assign_engines(net, enable_gp=False)
                em = Emitter(nc, wp, n_scratch=9)
                em.emit(net, final_out_ap=outt[:])

                for img in range(2):
                    p0 = img * NBLK
                    dst = bass.AP(y, img * IMG + w0 * C,
                                  [[R * ROW, NBLK], [ROW, R], [1, WSEL]])
                    nc.sync.dma_start(out=dst, in_=outt[p0:p0 + NBLK, :, :])

    nc.finalize()
    return nc


def build_chunk_net_real(xt):
    net = Net()
    leaves = [net.leaf(xt[:, d:d + R, :], WS) for d in range(5)]
    # inline build (same as build_chunk_net but with shared net)
    s = sort5(net, leaves, {i: f"s{i}" for i in range(5)})
    a = [net.view(s[i], 0, WPM) for i in range(5)]
    b = [net.view(s[i], C, WPM) for i in range(5)]
    pm = merge55(net, a, b, [f"pm{i}" for i in range(10)])
    Lv = [net.view(p, 0, WSEL) for p in pm]
    Rv = [net.view(p, 3 * C, WSEL) for p in pm]
    M = [net.view(s[i], 2 * C, WSEL) for i in range(5)]
    # o/e outputs and u reuse dead PM slots (odd slots die after the
    # o-partial internals, even slots after the e-partial internals)
    o3, o4, o5 = m55_mid_partial(net, Lv[1::2], Rv[1::2], "o",
                                 ["pm1", "pm3", "pm5"])
    e4, e5, e6 = m55_mid_partial(net, Lv[0::2], Rv[0::2], "e",
                                 ["pm7", "pm9", "pm0"])
    u0 = net.MIN(o3, e4, "pm2")
    u1 = net.MAX(o3, e4, "pm4")
    u2 = net.MIN(o4, e5, "pm6")
    u3 = net.MAX(o4, e5, "pm8")
    u4 = net.MIN(o5, e6)
    u5 = net.MAX(o5, e6)
    q0 = net.MIN(u3, M[3])
    p1 = net.MIN(net.MAX(u1, M[1]), u5)
    o2p = net.MAX(q0, p1)
    k1p = net.MAX(u2, M[2])
    h2p = net.MAX(net.MAX(u0, M[0]), net.MIN(u4, M[4]))
    e3p = net.MIN(k1p, h2p)
    net.MIN(o2p, e3p)
    return net


def find_ce_pairs(net):
    """Detect (min, max) node pairs on identical operands (CE pairs).
    Returns dict node -> partner (both directions)."""
    pairs = {}
    by_key = {}
    for v in net.nodes:
        key = (id(v.a), id(v.b))
        if key in by_key:
            u = by_key[key]
            if u.op != v.op and u not in pairs:
                pairs[u] = v
                pairs[v] = u
                continue
        by_key[key] = v
    return pairs


def assign_engines(net, pair_gp=3.61, single_gp=5.13, enable_gp=True):
    """Greedy two-engine list scheduling over schedulable units (CE pairs
    merged). Costs in DVE-op units. Mutates node.eng."""
    pairs = find_ce_pairs(net)
    avail = {"v": 0.0, "g": 0.0}
    done = {}

    def ready(v):
        if v.kind == "leaf":
            return 0.0
        if v.kind == "view":
            return ready(v.parent)
        return done[v]

    seen = set()
    for v in net.nodes:
        if v in seen:
            continue
        partner = pairs.get(v)
        if partner is not None:
            unit = (v, partner)
            cost_v, cost_g = 2.0, pair_gp
        else:
            unit = (v,)
            cost_v, cost_g = 1.0, single_gp
        dep = 0.0
        for u in unit:
            dep = max(dep, ready(u.a), ready(u.b))
        fin_v = max(avail["v"], dep) + cost_v
        fin_g = max(avail["g"], dep) + cost_g
        if enable_gp and fin_g < fin_v:
            eng, fin = "g", fin_g
        else:
            eng, fin = "v", fin_v
        for u in unit:
            u.eng = eng
            done[u] = fin
            seen.add(u)
        avail[eng] = fin
    return avail


_NC = None


def _get_nc():
    global _NC
    if _NC is None:
        _NC = build_nc()
    return _NC


def kernel(x, k):
    assert int(k) == 5
    x = np.ascontiguousarray(np.asarray(x, dtype=np.float32))
    assert x.shape == (16, H, W, C)
    nc = _get_nc()
    in_maps = [{"x": x[2 * i:2 * i + 2]} for i in range(8)]
    res = run_bass_kernel_spmd(nc, in_maps, core_ids=list(range(8)))
    return np.concatenate([r["out"] for r in res.results], axis=0)
